# revision 34
# baseline (speedup 1.0000x reference)
"""Mat2Twist Trainium2 kernel: batch of 3x3 rotation matrices -> twist vectors.

For each matrix R:  tr = trace(R); x = (tr-1)/2 = cos(theta)
  theta = arccos(x) = pi/2 - arctan(x / sqrt(1 - x^2))
  w = [R21-R12, R02-R20, R10-R01]   (unnormalized axis, |w| = 2 sin theta)
  out = theta/(2 sin theta) * w

Data-parallel over 8 NeuronCores. The host pre-arranges each core's shard
tile-major/component-major (PERM) so every on-chip op and DMA is unit-stride.
Per chunk (m matrices per partition), tile X = [minu(3m)|subt(3m)|R00|R11|R22]:
  tr   = R00 + R11 + R22                           [GpSimd TT x2]
  v    = Square(0.5*tr - 0.5)     = x^2            [ACT]
  lg   = Ln(1 - v)                                 [ACT]
  r    = Exp(-0.5*lg)             = 1/sin theta    [ACT]
  w    = X[0:3m] - X[3m:6m]       in place         [DVE TT]
  xr   = (tr - 1) * r             = 2 cot theta    [DVE STT]
  t    = Arctan(0.5*xr)           = pi/2 - theta   [ACT, in place]
  msc2 = (t - pi/2) * r           = -theta/sin th  [DVE STT]
  out_k = (-0.5*msc2) * w_k                        [DVE STT x3, in place]

All engines are in-order queues, so emission is SOFTWARE-PIPELINED with a
per-stage chunk skew:
  dma(i)@i -> trace adds@i+1 -> sq/ln/exp + w-sub@i+2 -> xr,arctan@i+3 ->
  msc2/fused-mul@i+4 -> out-DMA trigger@i+5 (emitted at the top of the
  iteration, a full iteration after the muls, so the ACT-HWDGE ring trigger
  never blocks the ACT queue).
Inputs ride the SP ring (~340+ GB/s sustained), outputs the ACT ring in
parallel. Tail chunks are half-size to shorten the pipeline drain. The three
final multiplies are ONE scalar_tensor_tensor over [P,3,m] with msc2
broadcast via a stride-0 dim.
"""

import numpy as np

import concourse.bass as bass
import concourse.mybir as mybir
from concourse.tile import TileContext
from concourse.bass_utils import run_bass_kernel_spmd

B = 4194304
NCORES = 8
P = 128
N_C = B // NCORES        # 524288 matrices per core
MPP = N_C // P           # 4096 matrices per partition
MS = [512] * 7 + [256] * 2   # per-chunk matrices per partition
assert sum(MS) == MPP

# component order in DRAM (flat 3x3 index): minuends, subtrahends, diagonal
PERM = [7, 2, 3, 5, 6, 1, 0, 4, 8]

F32 = mybir.dt.float32
ACT = mybir.ActivationFunctionType
ALU = mybir.AluOpType
PI_2 = float(np.pi / 2.0)
MAXM = max(MS)


def _split_multi_waits(nc):
    """This container's walrus build rejects >1 sem-wait per instruction
    ("Too many sync wait commands"); hoist extras onto preceding NOPs."""
    for f in nc.m.functions:
        for blk in f.blocks:
            il = blk.instructions
            new = []
            for ins in il:
                si = ins.sync_info
                if si is not None and si.on_wait is not None and len(si.on_wait) > 1:
                    waits = list(si.on_wait)
                    for j, w in enumerate(waits[:-1]):
                        nop = mybir.InstNoOp(name=f"{ins.name}-ws{j}", engine=ins.engine)
                        nop.sync_info = mybir.SyncInfo(on_wait=[w], on_update=[])
                        new.append(nop)
                    ins.sync_info = mybir.SyncInfo(
                        on_wait=[waits[-1]], on_update=list(si.on_update or [])
                    )
                new.append(ins)
            il[:] = new


def _register_const_ap(nc, value):
    """Mimic Bass.__init__'s register_const_ap for an extra f32 constant
    (memset + barrier happen before TileContext, same as the built-ins)."""
    tensor = nc.alloc_sbuf_tensor(f"const-f32-{value}", [128, 1], F32)
    nc.gpsimd.memset(tensor.ap(), value)
    nc.const_aps.aps[(F32, value)] = tensor.ap()
    nc.all_engine_barrier()


def _build_kernel():
    nc = bass.Bass()
    _register_const_ap(nc, -0.5)
    x_in = nc.dram_tensor("mat_in", [N_C * 9], F32, kind="ExternalInput")
    y_out = nc.dram_tensor("twist_out", [N_C * 3], F32, kind="ExternalOutput")

    n = len(MS)
    offs = [0] + list(np.cumsum(MS)[:-1].astype(int))

    with TileContext(nc) as tc:
        with tc.tile_pool(name="xp", bufs=8) as xp, \
             tc.tile_pool(name="tlong", bufs=5) as tlong, \
             tc.tile_pool(name="tshort", bufs=3) as tshort:

            X_, tr_, r_, xr_ = {}, {}, {}, {}

            def dma_in(i):
                m = MS[i]
                base = offs[i] * P * 9
                X_[i] = xp.tile([P, 9 * MAXM], F32, tag="X", name=f"X{i}")[:, : 9 * m]
                nc.sync.dma_start(
                    out=X_[i],
                    in_=x_in[base : base + P * 9 * m].rearrange("(p n) -> p n", p=P),
                )

            def gp_trace(i):
                m = MS[i]
                X = X_[i]
                tr = tlong.tile([P, MAXM], F32, tag="tr", name=f"tr{i}")[:, :m]
                nc.gpsimd.tensor_add(
                    out=tr, in0=X[:, 6 * m : 7 * m], in1=X[:, 7 * m : 8 * m]
                )
                nc.gpsimd.tensor_add(out=tr, in0=tr, in1=X[:, 8 * m : 9 * m])
                tr_[i] = tr

            def act_lnexp(i):
                m = MS[i]
                tr = tr_[i]
                v = tshort.tile([P, MAXM], F32, tag="v", name=f"v{i}")[:, :m]
                nc.scalar.activation(v, tr, ACT.Square, scale=0.5, bias=-0.5)
                lg = tshort.tile([P, MAXM], F32, tag="lg", name=f"lg{i}")[:, :m]
                nc.scalar.activation(lg, v, ACT.Ln, bias=1.0, scale=-1.0)
                r = tlong.tile([P, MAXM], F32, tag="r", name=f"r{i}")[:, :m]
                nc.scalar.activation(r, lg, ACT.Exp, scale=-0.5)
                r_[i] = r

            def dve_xr(i):
                m = MS[i]
                xr = tlong.tile([P, MAXM], F32, tag="xr", name=f"xr{i}")[:, :m]
                nc.vector.scalar_tensor_tensor(
                    out=xr, in0=tr_[i], scalar=1.0, in1=r_[i],
                    op0=ALU.subtract, op1=ALU.mult,
                )
                xr_[i] = xr

            def act_arctan(i):
                nc.scalar.activation(xr_[i], xr_[i], ACT.Arctan, scale=0.5)

            def dve_sub(i):
                m = MS[i]
                X = X_[i]
                nc.vector.tensor_sub(
                    out=X[:, 0 : 3 * m], in0=X[:, 0 : 3 * m], in1=X[:, 3 * m : 6 * m]
                )

            def dve_out(i):
                m = MS[i]
                X = X_[i]
                msc2 = tshort.tile([P, MAXM], F32, tag="msc2", name=f"msc2{i}")[:, :m]
                nc.vector.scalar_tensor_tensor(
                    out=msc2, in0=xr_[i], scalar=PI_2, in1=r_[i],
                    op0=ALU.subtract, op1=ALU.mult,
                )
                # one STT over all 3 w-blocks with msc2 broadcast (stride-0 dim)
                bcast = msc2.rearrange("p (o n) -> p o n", o=1).to_broadcast((P, 3, m))
                w3 = X[:, 0 : 3 * m].rearrange("p (k n) -> p k n", k=3)
                nc.vector.scalar_tensor_tensor(
                    out=w3, in0=bcast, scalar=-0.5, in1=w3,
                    op0=ALU.mult, op1=ALU.mult,
                )

            def out_dma(i):
                m = MS[i]
                dst = y_out[offs[i] * P * 3 : (offs[i] + m) * P * 3].rearrange(
                    "(p n) -> p n", p=P
                )
                nc.scalar.dma_start(out=dst, in_=X_[i][:, 0 : 3 * m])

            def valid(j):
                return 0 <= j < n

            # software-pipelined emission; skew in iterations:
            # dma(i)@i, trace@i+1, lnexp+sub@i+2, xr/arctan@i+3,
            # msc2/muls@i+4, out-dma@i+5 (trigger emitted at the top of the
            # iteration AFTER the muls, so the ACT-ring trigger never waits)
            for i in range(n + 5):
                if valid(i):
                    dma_in(i)
                if valid(i - 5):
                    out_dma(i - 5)
                if valid(i - 1):
                    gp_trace(i - 1)
                if valid(i - 3):
                    dve_xr(i - 3)
                if valid(i - 2):
                    act_lnexp(i - 2)
                if valid(i - 3):
                    act_arctan(i - 3)
                if valid(i - 2):
                    dve_sub(i - 2)
                if valid(i - 4):
                    dve_out(i - 4)

    _split_multi_waits(nc)
    return nc


_NC_CACHE = []


def _host_pack(mat_batch: np.ndarray) -> np.ndarray:
    """[B,3,3] -> [NCORES, N_C*9] tile-major/component-major PERM layout."""
    flat = np.ascontiguousarray(mat_batch, dtype=np.float32).reshape(
        NCORES, N_C, 9
    )
    out = np.empty((NCORES, N_C * 9), np.float32)
    pos = 0
    for m, off in zip(MS, np.concatenate([[0], np.cumsum(MS)[:-1]])):
        off = int(off)
        chunk = flat[:, off * P : (off + m) * P, :].reshape(NCORES, P, m, 9)
        sz = P * m * 9
        out[:, pos : pos + sz] = (
            chunk.transpose(0, 1, 3, 2)[:, :, PERM, :].reshape(NCORES, sz)
        )
        pos += sz
    return out


def _host_unpack(res_list) -> np.ndarray:
    out = np.empty((B, 3), np.float32)
    o = out.reshape(NCORES, N_C, 3)
    for i, r in enumerate(res_list):
        y = r["twist_out"]
        pos = 0
        for m, off in zip(MS, np.concatenate([[0], np.cumsum(MS)[:-1]])):
            off = int(off)
            sz = P * m * 3
            blk = y[pos : pos + sz].reshape(P, 3, m)
            o[i, off * P : (off + m) * P, :] = blk.transpose(0, 2, 1).reshape(
                P * m, 3
            )
            pos += sz
    return out


def kernel(mat_batch: np.ndarray) -> np.ndarray:
    if not _NC_CACHE:
        _NC_CACHE.append(_build_kernel())
    nc = _NC_CACHE[0]

    packed = _host_pack(mat_batch)
    in_maps = [{"mat_in": packed[i]} for i in range(NCORES)]
    res = run_bass_kernel_spmd(nc, in_maps, core_ids=list(range(NCORES)))
    return _host_unpack(res.results)



# revision 35
# speedup vs baseline: 1.1172x; 1.1172x over previous
"""Mat2Twist Trainium2 kernel: batch of 3x3 rotation matrices -> twist vectors.

For each matrix R:  tr = trace(R); x = (tr-1)/2 = cos(theta)
  theta = arccos(x) = pi/2 - arctan(x / sqrt(1 - x^2))
  w = [R21-R12, R02-R20, R10-R01]   (unnormalized axis, |w| = 2 sin theta)
  out = theta/(2 sin theta) * w

Data-parallel over 8 NeuronCores. The host pre-arranges each core's shard
tile-major/component-major (PERM) so every on-chip op and DMA is unit-stride.
Per chunk (m matrices per partition), tile X = [minu(3m)|subt(3m)|R00|R11|R22]:
  tr   = R00 + R11 + R22                           [GpSimd TT x2]
  v    = Square(0.5*tr - 0.5)     = x^2            [ACT]
  lg   = Ln(1 - v)                                 [ACT]
  r    = Exp(-0.5*lg)             = 1/sin theta    [ACT]
  w    = X[0:3m] - X[3m:6m]       in place         [DVE TT]
  xr   = (tr - 1) * r             = 2 cot theta    [DVE STT]
  t    = Arctan(0.5*xr)           = pi/2 - theta   [ACT, in place]
  msc2 = (t - pi/2) * r           = -theta/sin th  [DVE STT]
  out_k = (-0.5*msc2) * w_k                        [DVE STT x3, in place]

All engines are in-order queues, so emission is SOFTWARE-PIPELINED with a
per-stage chunk skew:
  dma(i)@i -> trace adds@i+1 -> sq/ln/exp + w-sub@i+2 -> xr,arctan@i+3 ->
  msc2/fused-mul@i+4 -> out-DMA trigger@i+5 (emitted at the top of the
  iteration, a full iteration after the muls, so the ACT-HWDGE ring trigger
  never blocks the ACT queue).
Inputs ride the SP ring (~340+ GB/s sustained), outputs the ACT ring in
parallel. Tail chunks are half-size to shorten the pipeline drain. The three
final multiplies are ONE scalar_tensor_tensor over [P,3,m] with msc2
broadcast via a stride-0 dim.
"""

import numpy as np

import concourse.bass as bass
import concourse.mybir as mybir
from concourse.tile import TileContext
from concourse.bass_utils import run_bass_kernel_spmd

B = 4194304
NCORES = 8
P = 128
N_C = B // NCORES        # 524288 matrices per core
MPP = N_C // P           # 4096 matrices per partition
MS = [512] * 7 + [384, 128]   # per-chunk matrices per partition
assert sum(MS) == MPP

# component order in DRAM (flat 3x3 index): minuends, subtrahends, diagonal
PERM = [7, 2, 3, 5, 6, 1, 0, 4, 8]

F32 = mybir.dt.float32
ACT = mybir.ActivationFunctionType
ALU = mybir.AluOpType
PI_2 = float(np.pi / 2.0)
MAXM = max(MS)


def _split_multi_waits(nc):
    """This container's walrus build rejects >1 sem-wait per instruction
    ("Too many sync wait commands"); hoist extras onto preceding NOPs."""
    for f in nc.m.functions:
        for blk in f.blocks:
            il = blk.instructions
            new = []
            for ins in il:
                si = ins.sync_info
                if si is not None and si.on_wait is not None and len(si.on_wait) > 1:
                    waits = list(si.on_wait)
                    for j, w in enumerate(waits[:-1]):
                        nop = mybir.InstNoOp(name=f"{ins.name}-ws{j}", engine=ins.engine)
                        nop.sync_info = mybir.SyncInfo(on_wait=[w], on_update=[])
                        new.append(nop)
                    ins.sync_info = mybir.SyncInfo(
                        on_wait=[waits[-1]], on_update=list(si.on_update or [])
                    )
                new.append(ins)
            il[:] = new


def _register_const_ap(nc, value):
    """Mimic Bass.__init__'s register_const_ap for an extra f32 constant
    (memset + barrier happen before TileContext, same as the built-ins)."""
    tensor = nc.alloc_sbuf_tensor(f"const-f32-{value}", [128, 1], F32)
    nc.gpsimd.memset(tensor.ap(), value)
    nc.const_aps.aps[(F32, value)] = tensor.ap()
    nc.all_engine_barrier()


def _build_kernel():
    nc = bass.Bass()
    _register_const_ap(nc, -0.5)
    x_in = nc.dram_tensor("mat_in", [N_C * 9], F32, kind="ExternalInput")
    y_out = nc.dram_tensor("twist_out", [N_C * 3], F32, kind="ExternalOutput")

    n = len(MS)
    offs = [0] + list(np.cumsum(MS)[:-1].astype(int))

    with TileContext(nc) as tc:
        with tc.tile_pool(name="xp", bufs=8) as xp, \
             tc.tile_pool(name="tlong", bufs=5) as tlong, \
             tc.tile_pool(name="tshort", bufs=3) as tshort:

            X_, tr_, r_, xr_ = {}, {}, {}, {}

            def dma_in(i):
                m = MS[i]
                base = offs[i] * P * 9
                X_[i] = xp.tile([P, 9 * MAXM], F32, tag="X", name=f"X{i}")[:, : 9 * m]
                nc.sync.dma_start(
                    out=X_[i],
                    in_=x_in[base : base + P * 9 * m].rearrange("(p n) -> p n", p=P),
                )

            def gp_trace(i):
                m = MS[i]
                X = X_[i]
                tr = tlong.tile([P, MAXM], F32, tag="tr", name=f"tr{i}")[:, :m]
                nc.gpsimd.tensor_add(
                    out=tr, in0=X[:, 6 * m : 7 * m], in1=X[:, 7 * m : 8 * m]
                )
                nc.gpsimd.tensor_add(out=tr, in0=tr, in1=X[:, 8 * m : 9 * m])
                tr_[i] = tr

            def act_lnexp(i):
                m = MS[i]
                tr = tr_[i]
                v = tshort.tile([P, MAXM], F32, tag="v", name=f"v{i}")[:, :m]
                nc.scalar.activation(v, tr, ACT.Square, scale=0.5, bias=-0.5)
                lg = tshort.tile([P, MAXM], F32, tag="lg", name=f"lg{i}")[:, :m]
                nc.scalar.activation(lg, v, ACT.Ln, bias=1.0, scale=-1.0)
                r = tlong.tile([P, MAXM], F32, tag="r", name=f"r{i}")[:, :m]
                nc.scalar.activation(r, lg, ACT.Exp, scale=-0.5)
                r_[i] = r

            def dve_xr(i):
                m = MS[i]
                xr = tlong.tile([P, MAXM], F32, tag="xr", name=f"xr{i}")[:, :m]
                nc.vector.scalar_tensor_tensor(
                    out=xr, in0=tr_[i], scalar=1.0, in1=r_[i],
                    op0=ALU.subtract, op1=ALU.mult,
                )
                xr_[i] = xr

            def act_arctan(i):
                nc.scalar.activation(xr_[i], xr_[i], ACT.Arctan, scale=0.5)

            def dve_sub(i):
                m = MS[i]
                X = X_[i]
                nc.vector.tensor_sub(
                    out=X[:, 0 : 3 * m], in0=X[:, 0 : 3 * m], in1=X[:, 3 * m : 6 * m]
                )

            def dve_out(i):
                m = MS[i]
                X = X_[i]
                msc2 = tshort.tile([P, MAXM], F32, tag="msc2", name=f"msc2{i}")[:, :m]
                nc.vector.scalar_tensor_tensor(
                    out=msc2, in0=xr_[i], scalar=PI_2, in1=r_[i],
                    op0=ALU.subtract, op1=ALU.mult,
                )
                # one STT over all 3 w-blocks with msc2 broadcast (stride-0 dim)
                bcast = msc2.rearrange("p (o n) -> p o n", o=1).to_broadcast((P, 3, m))
                w3 = X[:, 0 : 3 * m].rearrange("p (k n) -> p k n", k=3)
                nc.vector.scalar_tensor_tensor(
                    out=w3, in0=bcast, scalar=-0.5, in1=w3,
                    op0=ALU.mult, op1=ALU.mult,
                )

            def out_dma(i):
                m = MS[i]
                dst = y_out[offs[i] * P * 3 : (offs[i] + m) * P * 3].rearrange(
                    "(p n) -> p n", p=P
                )
                nc.scalar.dma_start(out=dst, in_=X_[i][:, 0 : 3 * m])

            def valid(j):
                return 0 <= j < n

            # software-pipelined emission; skew in iterations:
            # dma(i)@i, trace@i+1, lnexp+sub@i+2, xr/arctan@i+3,
            # msc2/muls@i+4, out-dma@i+5 (trigger emitted at the top of the
            # iteration AFTER the muls, so the ACT-ring trigger never waits)
            for i in range(n + 5):
                if valid(i):
                    dma_in(i)
                if valid(i - 5):
                    out_dma(i - 5)
                if valid(i - 1):
                    gp_trace(i - 1)
                if valid(i - 3):
                    dve_xr(i - 3)
                if valid(i - 2):
                    act_lnexp(i - 2)
                if valid(i - 3):
                    act_arctan(i - 3)
                if valid(i - 2):
                    dve_sub(i - 2)
                if valid(i - 4):
                    dve_out(i - 4)

    _split_multi_waits(nc)
    return nc


_NC_CACHE = []


def _host_pack(mat_batch: np.ndarray) -> np.ndarray:
    """[B,3,3] -> [NCORES, N_C*9] tile-major/component-major PERM layout."""
    flat = np.ascontiguousarray(mat_batch, dtype=np.float32).reshape(
        NCORES, N_C, 9
    )
    out = np.empty((NCORES, N_C * 9), np.float32)
    pos = 0
    for m, off in zip(MS, np.concatenate([[0], np.cumsum(MS)[:-1]])):
        off = int(off)
        chunk = flat[:, off * P : (off + m) * P, :].reshape(NCORES, P, m, 9)
        sz = P * m * 9
        out[:, pos : pos + sz] = (
            chunk.transpose(0, 1, 3, 2)[:, :, PERM, :].reshape(NCORES, sz)
        )
        pos += sz
    return out


def _host_unpack(res_list) -> np.ndarray:
    out = np.empty((B, 3), np.float32)
    o = out.reshape(NCORES, N_C, 3)
    for i, r in enumerate(res_list):
        y = r["twist_out"]
        pos = 0
        for m, off in zip(MS, np.concatenate([[0], np.cumsum(MS)[:-1]])):
            off = int(off)
            sz = P * m * 3
            blk = y[pos : pos + sz].reshape(P, 3, m)
            o[i, off * P : (off + m) * P, :] = blk.transpose(0, 2, 1).reshape(
                P * m, 3
            )
            pos += sz
    return out


def kernel(mat_batch: np.ndarray) -> np.ndarray:
    if not _NC_CACHE:
        _NC_CACHE.append(_build_kernel())
    nc = _NC_CACHE[0]

    packed = _host_pack(mat_batch)
    in_maps = [{"mat_in": packed[i]} for i in range(NCORES)]
    res = run_bass_kernel_spmd(nc, in_maps, core_ids=list(range(NCORES)))
    return _host_unpack(res.results)

